# revision 1
# baseline (speedup 1.0000x reference)
"""Fused transformer block (RMSNorm + qk-norm attention + MLP) for TRN2, 8 cores.

Sharding: 8 cores = (4 batches) x (2 query-halves). Each core gets its batch's
full sequence with rows rotated so its query half is rows 0..1023 (attention is
permutation-invariant over keys, so K/V row order doesn't matter). No
collectives needed; each core produces a disjoint [1024, 768] output slice.

Layout strategy per core:
  - x_hat = rmsnorm(lat) in natural [s, d] layout, cast bf16, round-tripped
    through DRAM with chunked DMA-transposes (512-row chunks pipeline with
    producers/consumers) to get x_hat^T [d, s] for the projections.
  - Q/K projections in natural layout (lhsT = x_hat^T tile), qk-rmsnorm applied
    in natural layout, then DMA-transposed to Q^T/K^T [hd, s] per head pair.
  - logits^T[k, q] = K^T_h.T @ Q^T_h; head pairs issue to disjoint PE row
    groups (partition base 0/64) so both heads' logits matmuls overlap.
    exp on ScalarE with no max subtraction (|logit| <= 8: q, k are unit-RMS).
    P[k, q] feeds attn@V with V in natural [k, hd] layout augmented with a
    ones column -> softmax denominator lands in PSUM row 64 for free.
  - softmax division is decoupled from the PE stream: accumulators are
    evacuated to SBUF, reciprocals run on VectorE under the ACT-bound inner
    loop, and the ones-outer-product broadcasts + multiplies run at phase end.
  - out-proj / MLP2 use the activation tile as the stationary operand so the
    result comes out in natural [q, d] layout for residuals.
"""

import numpy as np
from contextlib import ExitStack

import concourse.bass as bass
import concourse.tile as tile
from concourse import bacc, mybir
from concourse.bass_utils import run_bass_kernel_spmd

F32 = mybir.dt.float32
BF16 = mybir.dt.bfloat16
AF = mybir.ActivationFunctionType
OP = mybir.AluOpType

B, S, D, H, HD, MLP = 4, 2048, 768, 12, 64, 3072
SQ = S // 2            # query rows per core
NT_S = S // 128        # 16 sequence tiles
NT_Q = SQ // 128       # 8 query tiles
NT_D = D // 128        # 6 model-dim tiles
NT_M = MLP // 128      # 24 mlp-dim tiles
EPS = 1e-6
VW = HD + 1            # V width incl. ones column
CH = 512               # transpose chunk (rows)
NC_S = S // CH         # 4 chunks over full sequence
NC_Q = SQ // CH        # 2 chunks over query rows


def _chunks(n):
    out, ofs = [], 0
    while ofs < n:
        c = min(512, n - ofs)
        out.append((ofs, c))
        ofs += c
    return out


def build_nc(sim_compat=False):
    nc = bacc.Bacc("TRN2", target_bir_lowering=False, debug=False, num_devices=8)

    lat = nc.dram_tensor("lat", [S, D], F32, kind="ExternalInput").ap()
    wq = nc.dram_tensor("wq", [D, D], BF16, kind="ExternalInput").ap()
    wk = nc.dram_tensor("wk", [D, D], BF16, kind="ExternalInput").ap()
    wv = nc.dram_tensor("wv", [D, D], BF16, kind="ExternalInput").ap()
    wo = nc.dram_tensor("wo", [D, D], BF16, kind="ExternalInput").ap()
    wi = nc.dram_tensor("wi", [D, MLP], BF16, kind="ExternalInput").ap()
    wom = nc.dram_tensor("wom", [MLP, D], BF16, kind="ExternalInput").ap()
    kqsc = nc.dram_tensor("kqsc", [128, 1], F32, kind="ExternalInput").ap()
    out = nc.dram_tensor("out", [SQ, D], F32, kind="ExternalOutput").ap()

    with tile.TileContext(nc) as tc, ExitStack() as top:
        def ptile(pool, shape, dtype, name):
            return pool.tile(shape, dtype, name=name, tag=name)

        p_const = top.enter_context(tc.tile_pool(name="p_const", bufs=1))
        p_x2 = top.enter_context(tc.tile_pool(name="p_x2", bufs=1))
        p_oT = tc.alloc_tile_pool(name="p_oT", bufs=1)
        p_att = tc.alloc_tile_pool(name="p_att", bufs=1)

        # ---- persistent tiles ----
        Vaug = ptile(p_att, [128, NT_S * H * VW], BF16, name="Vaug")
        oT = ptile(p_oT, [128, NT_D * SQ], BF16, name="oT")
        kqsc_t = ptile(p_const, [128, 1], F32, name="kqsc_t")
        onesF = ptile(p_const, [128, 64], F32, name="onesF")
        eps_t = ptile(p_const, [128, 1], F32, name="eps_t")
        KT = [ptile(p_att, [128, S], BF16, name=f"KT{d}") for d in range(NT_D)]
        QT = [ptile(p_att, [128, SQ], BF16, name=f"QT{d}") for d in range(NT_D)]
        x2 = [ptile(p_x2, [128, D], F32, name=f"x2_{q}") for q in range(NT_Q)]
        x2T = [ptile(p_x2, [128, SQ], BF16, name=f"x2T{d}") for d in range(NT_D)]

        nc.sync.dma_start(kqsc_t[:], kqsc[:])
        nc.vector.memset(eps_t[:], EPS)
        nc.vector.memset(onesF[:], 1.0)
        vview = Vaug[:].rearrange("p (s h k) -> p s h k", s=NT_S, h=H)
        nc.vector.memset(vview[:, :, :, HD:VW], 1.0)

        dram = top.enter_context(tc.tile_pool(name="dram", bufs=1, space="DRAM"))
        xh_d = dram.tile([S, D], BF16, name="xh_d")
        kh_d = dram.tile([S, D], BF16, name="kh_d")
        qh_d = dram.tile([SQ, D], BF16, name="qh_d")
        x2h_d = dram.tile([SQ, D], BF16, name="x2h_d")

        # =============== Phase A: ln1 + x_hat^T ===============
        p_xT = tc.alloc_tile_pool(name="p_xT", bufs=1)
        xT = [ptile(p_xT, [128, S], BF16, name=f"xT{d}") for d in range(NT_D)]
        with ExitStack() as ctx:
            io = ctx.enter_context(tc.tile_pool(name="a_io", bufs=5))
            st_p = ctx.enter_context(tc.tile_pool(name="a_stats", bufs=8))
            scr = ctx.enter_context(tc.tile_pool(name="a_scr", bufs=5))
            for t in range(NT_S):
                lt = io.tile([128, D], F32, name="lt")
                nc.sync.dma_start(lt[:], lat[t * 128:(t + 1) * 128, :])
                sq = scr.tile([128, D], F32, name="sq")
                ssq = st_p.tile([128, 1], F32, name="ssq")
                nc.scalar.activation(sq[:], lt[:], AF.Square, accum_out=ssq[:])
                srt = st_p.tile([128, 1], F32, name="srt")
                nc.scalar.activation(srt[:], ssq[:], AF.Sqrt, bias=eps_t[:], scale=1.0 / D)
                rs = st_p.tile([128, 1], F32, name="rs")
                nc.vector.reciprocal(rs[:], srt[:])
                xh = scr.tile([128, D], BF16, name="xh")
                nc.vector.tensor_scalar_mul(xh[:], lt[:], rs[:])
                nc.gpsimd.dma_start(xh_d[t * 128:(t + 1) * 128, :], xh[:])
        for d in range(NT_D):
            nc.sync.dma_start_transpose(xT[d][:], xh_d[:, d * 128:(d + 1) * 128])

        # =============== Phase B: Q/K/V projections + qk-norm ===============
        with ExitStack() as ctx:
            wp = ctx.enter_context(tc.tile_pool(name="b_w", bufs=1))
            wq_sb = [wp.tile([128, D], BF16, name=f"wq_sb{d}") for d in range(NT_D)]
            wk_sb = [wp.tile([128, D], BF16, name=f"wk_sb{d}") for d in range(NT_D)]
            wv_sb = [wp.tile([128, D], BF16, name=f"wv_sb{d}") for d in range(NT_D)]
            for d in range(NT_D):
                nc.sync.dma_start(wq_sb[d][:], wq[d * 128:(d + 1) * 128, :])
                nc.sync.dma_start(wk_sb[d][:], wk[d * 128:(d + 1) * 128, :])
                nc.sync.dma_start(wv_sb[d][:], wv[d * 128:(d + 1) * 128, :])

            ps = ctx.enter_context(tc.tile_pool(name="b_ps", bufs=3, space="PSUM"))
            scr = ctx.enter_context(tc.tile_pool(name="b_scr", bufs=3))
            st_p = ctx.enter_context(tc.tile_pool(name="b_stats", bufs=6))
            natp = ctx.enter_context(tc.tile_pool(name="b_nat", bufs=3))

            def proj(t, w_sb):
                p = ps.tile([128, D], F32, name="p_proj")
                for d in range(NT_D):
                    lhsT = xT[d][:, t * 128:(t + 1) * 128]
                    for ofs, n in _chunks(D):
                        nc.tensor.matmul(
                            p[:, ofs:ofs + n], lhsT, w_sb[d][:, ofs:ofs + n],
                            start=(d == 0), stop=(d == NT_D - 1))
                return p

            def qknorm(p, dst_dram, t):
                sq = scr.tile([128, D], F32, name="sq_b")
                nc.scalar.activation(sq[:], p[:], AF.Square)
                ss = st_p.tile([128, H], F32, name="ss_b")
                nc.vector.tensor_reduce(
                    ss[:], sq[:].rearrange("p (h k) -> p h k", h=H),
                    axis=mybir.AxisListType.X, op=OP.add)
                srt = st_p.tile([128, H], F32, name="srt_b")
                nc.scalar.activation(srt[:], ss[:], AF.Sqrt, bias=eps_t[:], scale=1.0 / HD)
                rs = st_p.tile([128, H], F32, name="rs_b")
                nc.vector.reciprocal(rs[:], srt[:])
                nat = natp.tile([128, D], BF16, name="nat_b")
                rs_view = rs[:].rearrange("p (h o) -> p h o", o=1).broadcast_to([128, H, HD])
                nc.vector.tensor_tensor(
                    out=nat[:].rearrange("p (h k) -> p h k", h=H),
                    in0=p[:].rearrange("p (h k) -> p h k", h=H),
                    in1=rs_view, op=OP.mult)
                nc.gpsimd.dma_start(dst_dram[t * 128:(t + 1) * 128, :], nat[:])

            for t in range(NT_S):
                pk = proj(t, wk_sb)
                qknorm(pk, kh_d, t)
                pv = proj(t, wv_sb)
                nc.vector.tensor_copy(
                    vview[:, t, :, 0:HD],
                    pv[:].rearrange("p (h k) -> p h k", h=H))
                if t < NT_Q:
                    pq = proj(t, wq_sb)
                    qknorm(pq, qh_d, t)
        for d in range(NT_D):
            nc.sync.dma_start_transpose(KT[d][:], kh_d[:, d * 128:(d + 1) * 128])
            nc.vector.tensor_scalar_mul(KT[d][:], KT[d][:], kqsc_t[:])
            nc.sync.dma_start_transpose(QT[d][:], qh_d[:, d * 128:(d + 1) * 128])
        p_xT.release()

        # =============== Phase C: attention ===============
        with ExitStack() as ctx:
            psL = ctx.enter_context(tc.tile_pool(name="c_psL", bufs=2, space="PSUM"))
            psO = ctx.enter_context(tc.tile_pool(name="c_psO", bufs=2, space="PSUM"))
            pp = ctx.enter_context(tc.tile_pool(name="c_p", bufs=6))
            oup = ctx.enter_context(tc.tile_pool(name="c_oU", bufs=5))

            def divide_head(h, oU_h):
                # broadcast 1/denom (held in-place at partition 64 of oU_h)
                # across 64 partitions via ones outer product, then multiply.
                dt, base = h // 2, (h % 2) * 64
                b_ps = psL.tile([64, SQ], F32, name="b_ps", tag="l_ps")
                for ofs, n in _chunks(SQ):
                    nc.tensor.matmul(b_ps[:, ofs:ofs + n], onesF[64:65, :],
                                     oU_h[VW - 1:VW, ofs:ofs + n],
                                     start=True, stop=True)
                nc.vector.scalar_tensor_tensor(
                    oT[base:base + 64, dt * SQ:(dt + 1) * SQ],
                    b_ps[:], 1.0, oU_h[0:HD, :], op0=OP.bypass, op1=OP.mult)

            pending = []
            for hp in range(H // 2):
                dt = hp
                o_ps = [psO.tile([VW, SQ], F32, name=f"o_ps{e}", tag="o_ps")
                        for e in range(2)]
                for t in range(NT_S):
                    l_ps = [psL.tile([128, SQ], F32, name=f"l_ps{e}", tag="l_ps")
                            for e in range(2)]
                    for e in range(2):  # head 2*hp+e at PE row group 64*e
                        base = 64 * e
                        lhsT = KT[dt][base:base + 64, t * 128:(t + 1) * 128]
                        for j in range(NC_Q):
                            nc.tensor.matmul(
                                l_ps[e][:, j * CH:(j + 1) * CH], lhsT,
                                QT[dt][base:base + 64, j * CH:(j + 1) * CH],
                                start=True, stop=True)
                    p_t = [None, None]
                    for e in range(2):
                        p_t[e] = pp.tile([128, SQ], BF16, name=f"p_t{e}", tag="p_t")
                        nc.scalar.activation(p_t[e][:], l_ps[e][:], AF.Exp)
                    for e in range(2):
                        h = 2 * hp + e
                        vofs = t * H * VW + h * VW
                        for ofs, n in _chunks(SQ):
                            nc.tensor.matmul(
                                o_ps[e][:, ofs:ofs + n],
                                Vaug[:, vofs:vofs + VW],
                                p_t[e][:, ofs:ofs + n],
                                start=(t == 0), stop=(t == NT_S - 1))
                    if t == 4:
                        for h_prev, oU_prev in pending:
                            divide_head(h_prev, oU_prev)
                        pending = []
                for e in range(2):
                    h = 2 * hp + e
                    oU_h = oup.tile([VW, SQ], F32, name="oU", tag="oU")
                    nc.vector.tensor_copy(oU_h[:], o_ps[e][:])
                    nc.vector.reciprocal(oU_h[VW - 1:VW, :], oU_h[VW - 1:VW, :])
                    pending.append((h, oU_h))
            for h_prev, oU_prev in pending:
                divide_head(h_prev, oU_prev)
        p_att.release()

        # =============== Phase D: out-proj + residual + ln2 ===============
        with ExitStack() as ctx:
            wp = ctx.enter_context(tc.tile_pool(name="d_w", bufs=1))
            wo_sb = [wp.tile([128, D], BF16, name=f"wo_sb{d}") for d in range(NT_D)]
            for d in range(NT_D):
                nc.sync.dma_start(wo_sb[d][:], wo[d * 128:(d + 1) * 128, :])
            ps = ctx.enter_context(tc.tile_pool(name="d_ps", bufs=2, space="PSUM"))
            io = ctx.enter_context(tc.tile_pool(name="d_io", bufs=3))
            scr = ctx.enter_context(tc.tile_pool(name="d_scr", bufs=3))
            st_p = ctx.enter_context(tc.tile_pool(name="d_stats", bufs=4))

            for q in range(NT_Q):
                p = ps.tile([128, D], F32, name="p_oproj")
                for d in range(NT_D):
                    for ofs, n in _chunks(D):
                        nc.tensor.matmul(
                            p[:, ofs:ofs + n],
                            oT[:, d * SQ + q * 128: d * SQ + (q + 1) * 128],
                            wo_sb[d][:, ofs:ofs + n],
                            start=(d == 0), stop=(d == NT_D - 1))
                lt = io.tile([128, D], F32, name="lt_d")
                nc.sync.dma_start(lt[:], lat[q * 128:(q + 1) * 128, :])
                nc.vector.tensor_tensor(out=x2[q][:], in0=p[:], in1=lt[:], op=OP.add)
                sq = scr.tile([128, D], F32, name="sq_d")
                ssq = st_p.tile([128, 1], F32, name="ssq_d")
                nc.scalar.activation(sq[:], x2[q][:], AF.Square, accum_out=ssq[:])
                srt = st_p.tile([128, 1], F32, name="srt_d")
                nc.scalar.activation(srt[:], ssq[:], AF.Sqrt, bias=eps_t[:], scale=1.0 / D)
                rs = st_p.tile([128, 1], F32, name="rs_d")
                nc.vector.reciprocal(rs[:], srt[:])
                xh2 = scr.tile([128, D], BF16, name="xh2")
                nc.vector.tensor_scalar_mul(xh2[:], x2[q][:], rs[:])
                nc.gpsimd.dma_start(x2h_d[q * 128:(q + 1) * 128, :], xh2[:])
            for d in range(NT_D):
                nc.sync.dma_start_transpose(x2T[d][:], x2h_d[:, d * 128:(d + 1) * 128])
        p_oT.release()

        # =============== Phase E: MLP ===============
        p_hT = tc.alloc_tile_pool(name="p_hT", bufs=1)
        hT = ptile(p_hT, [128, NT_M * SQ], BF16, name="hT")
        with ExitStack() as ctx:
            wp = ctx.enter_context(tc.tile_pool(name="e_w", bufs=1))
            wi_sb = [wp.tile([128, MLP], BF16, name=f"wi_sb{d}") for d in range(NT_D)]
            for d in range(NT_D):
                nc.sync.dma_start(wi_sb[d][:], wi[d * 128:(d + 1) * 128, :])
            wom_sb = [wp.tile([128, D], BF16, name=f"wom_sb{m}") for m in range(NT_M)]
            for m in range(NT_M):
                nc.sync.dma_start(wom_sb[m][:], wom[m * 128:(m + 1) * 128, :])

            ps = ctx.enter_context(tc.tile_pool(name="e_ps", bufs=1, space="PSUM"))
            iop = ctx.enter_context(tc.tile_pool(name="e_io", bufs=3))

            for m in range(NT_M):
                p = ps.tile([128, SQ], F32, name="p_mlp1", bufs=2)
                for d in range(NT_D):
                    for j in range(NC_Q):
                        nc.tensor.matmul(
                            p[:, j * CH:(j + 1) * CH],
                            wi_sb[d][:, m * 128:(m + 1) * 128],
                            x2T[d][:, j * CH:(j + 1) * CH],
                            start=(d == 0), stop=(d == NT_D - 1))
                if not sim_compat:
                    nc.scalar.activation(hT[:, m * SQ:(m + 1) * SQ], p[:],
                                         AF.Gelu_apprx_tanh)
                else:
                    xsq = iop.tile([128, SQ], F32, name="g_xsq", bufs=1)
                    nc.vector.tensor_tensor(out=xsq[:], in0=p[:], in1=p[:], op=OP.mult)
                    w = iop.tile([128, SQ], F32, name="g_w", bufs=1)
                    nc.vector.tensor_scalar(w[:], xsq[:], 0.044715, 1.0,
                                            op0=OP.mult, op1=OP.add)
                    u = iop.tile([128, SQ], F32, name="g_u", bufs=1)
                    nc.vector.tensor_tensor(out=u[:], in0=w[:], in1=p[:], op=OP.mult)
                    th = iop.tile([128, SQ], F32, name="g_th", bufs=1)
                    nc.scalar.activation(th[:], u[:], AF.Tanh, scale=0.7978845608028654)
                    t2 = iop.tile([128, SQ], F32, name="g_t2", bufs=1)
                    nc.vector.scalar_tensor_tensor(t2[:], th[:], 1.0, p[:],
                                                   op0=OP.add, op1=OP.mult)
                    nc.vector.tensor_scalar_mul(hT[:, m * SQ:(m + 1) * SQ], t2[:], 0.5)

            for q in range(NT_Q):
                p = ps.tile([128, D], F32, name="p_mlp2", bufs=2)
                for m in range(NT_M):
                    for ofs, n in _chunks(D):
                        nc.tensor.matmul(
                            p[:, ofs:ofs + n],
                            hT[:, m * SQ + q * 128: m * SQ + (q + 1) * 128],
                            wom_sb[m][:, ofs:ofs + n],
                            start=(m == 0), stop=(m == NT_M - 1))
                ot = iop.tile([128, D], F32, name="ot_e")
                nc.vector.tensor_tensor(out=ot[:], in0=p[:], in1=x2[q][:], op=OP.add)
                nc.sync.dma_start(out[q * 128:(q + 1) * 128, :], ot[:])
        p_hT.release()

    nc.compile()
    return nc


def make_in_maps(latents, ln1_scale, wq, wk, wv, q_norm_scale, k_norm_scale,
                 wo_attn, ln2_scale, wi, wo_mlp):
    import ml_dtypes
    bf = ml_dtypes.bfloat16
    wq2 = (np.asarray(ln1_scale, np.float64)[:, None]
           * np.asarray(wq, np.float64).reshape(D, D)).astype(bf)
    wk2 = (np.asarray(ln1_scale, np.float64)[:, None]
           * np.asarray(wk, np.float64).reshape(D, D)).astype(bf)
    wv2 = (np.asarray(ln1_scale, np.float64)[:, None]
           * np.asarray(wv, np.float64).reshape(D, D)).astype(bf)
    wo2 = np.asarray(wo_attn, np.float32).reshape(D, D).astype(bf)
    wi2 = (np.asarray(ln2_scale, np.float64)[:, None]
           * np.asarray(wi, np.float64)).astype(bf)
    wom2 = np.asarray(wo_mlp, np.float32).astype(bf)
    kq = (np.tile(np.asarray(q_norm_scale, np.float64)
                  * np.asarray(k_norm_scale, np.float64), 2)
          / np.sqrt(HD)).astype(np.float32)[:, None]
    lat_np = np.asarray(latents, np.float32)
    in_maps = []
    for c in range(8):
        b, half = c // 2, c % 2
        lm = lat_np[b]
        lat_rot = np.concatenate([lm[half * SQ:(half + 1) * SQ],
                                  lm[(1 - half) * SQ:(2 - half) * SQ]], axis=0)
        in_maps.append(dict(lat=np.ascontiguousarray(lat_rot), wq=wq2, wk=wk2,
                            wv=wv2, wo=wo2, wi=wi2, wom=wom2, kqsc=kq))
    return in_maps


_NC_CACHE = None


def kernel(**inputs):
    global _NC_CACHE
    if _NC_CACHE is None:
        _NC_CACHE = build_nc()
    nc = _NC_CACHE
    in_maps = make_in_maps(**inputs)
    res = run_bass_kernel_spmd(nc, in_maps, list(range(8)))
    y = np.empty((B, S, D), np.float32)
    for c in range(8):
        b, half = c // 2, c % 2
        y[b, half * SQ:(half + 1) * SQ] = res.results[c]["out"]
    return y


if __name__ == "__main__":
    import reference
    inputs = {k: np.asarray(v) for k, v in reference.setup_inputs().items()}
    y = kernel(**inputs)
    exp = np.asarray(reference.reference(**reference.setup_inputs()))
    err = np.abs(y - exp).max() / np.abs(exp).max()
    print("Relative error:", err)



# revision 5
# speedup vs baseline: 1.1143x; 1.1143x over previous
"""Fused transformer block (RMSNorm + qk-norm attention + MLP) for TRN2, 8 cores.

Sharding: 8 cores = (4 batches) x (2 query-halves). Each core gets its batch's
full sequence with rows rotated so its query half is rows 0..1023 (attention is
permutation-invariant over keys, so K/V row order doesn't matter). No
collectives needed; each core produces a disjoint [1024, 768] output slice.

Layout strategy per core:
  - x_hat = rmsnorm(lat) in natural [s, d] layout, cast bf16, round-tripped
    through DRAM with chunked DMA-transposes (512-row chunks pipeline with
    producers/consumers) to get x_hat^T [d, s] for the projections.
  - Q/K projections in natural layout (lhsT = x_hat^T tile), qk-rmsnorm applied
    in natural layout, then DMA-transposed to Q^T/K^T [hd, s] per head pair.
  - logits^T[k, q] = K^T_h.T @ Q^T_h; head pairs issue to disjoint PE row
    groups (partition base 0/64) so both heads' logits matmuls overlap.
    exp on ScalarE with no max subtraction (|logit| <= 8: q, k are unit-RMS).
    P[k, q] feeds attn@V with V in natural [k, hd] layout augmented with a
    ones column -> softmax denominator lands in PSUM row 64 for free.
  - softmax division is decoupled from the PE stream: accumulators are
    evacuated to SBUF, reciprocals run on VectorE under the ACT-bound inner
    loop, and the ones-outer-product broadcasts + multiplies run at phase end.
  - out-proj / MLP2 use the activation tile as the stationary operand so the
    result comes out in natural [q, d] layout for residuals.
"""

import numpy as np
from contextlib import ExitStack

import concourse.bass as bass
import concourse.tile as tile
from concourse import bacc, mybir
from concourse.bass_utils import run_bass_kernel_spmd

F32 = mybir.dt.float32
BF16 = mybir.dt.bfloat16
AF = mybir.ActivationFunctionType
OP = mybir.AluOpType

B, S, D, H, HD, MLP = 4, 2048, 768, 12, 64, 3072
SQ = S // 2            # query rows per core
NT_S = S // 128        # 16 sequence tiles
NT_Q = SQ // 128       # 8 query tiles
NT_D = D // 128        # 6 model-dim tiles
NT_M = MLP // 128      # 24 mlp-dim tiles
EPS = 1e-6
VW = HD + 1            # V width incl. ones column
CH = 512               # transpose chunk (rows)
NC_S = S // CH         # 4 chunks over full sequence
NC_Q = SQ // CH        # 2 chunks over query rows


def _chunks(n):
    out, ofs = [], 0
    while ofs < n:
        c = min(512, n - ofs)
        out.append((ofs, c))
        ofs += c
    return out


def build_nc(sim_compat=False):
    nc = bacc.Bacc("TRN2", target_bir_lowering=False, debug=False, num_devices=8)

    lat = nc.dram_tensor("lat", [S, D], F32, kind="ExternalInput").ap()
    wq = nc.dram_tensor("wq", [D, D], BF16, kind="ExternalInput").ap()
    wk = nc.dram_tensor("wk", [D, D], BF16, kind="ExternalInput").ap()
    wv = nc.dram_tensor("wv", [D, D], BF16, kind="ExternalInput").ap()
    wo = nc.dram_tensor("wo", [D, D], BF16, kind="ExternalInput").ap()
    wi = nc.dram_tensor("wi", [D, MLP], BF16, kind="ExternalInput").ap()
    wom = nc.dram_tensor("wom", [MLP, D], BF16, kind="ExternalInput").ap()
    kqsc = nc.dram_tensor("kqsc", [128, 1], F32, kind="ExternalInput").ap()
    out = nc.dram_tensor("out", [SQ, D], F32, kind="ExternalOutput").ap()

    with tile.TileContext(nc) as tc, ExitStack() as top:
        def ptile(pool, shape, dtype, name):
            return pool.tile(shape, dtype, name=name, tag=name)

        p_const = top.enter_context(tc.tile_pool(name="p_const", bufs=1))
        p_x2 = top.enter_context(tc.tile_pool(name="p_x2", bufs=1))
        p_oT = tc.alloc_tile_pool(name="p_oT", bufs=1)
        p_att = tc.alloc_tile_pool(name="p_att", bufs=1)

        # ---- persistent tiles ----
        Vaug = ptile(p_att, [128, NT_S * H * VW], BF16, name="Vaug")
        oT = ptile(p_oT, [128, NT_D * SQ], BF16, name="oT")
        kqsc_t = ptile(p_const, [128, 1], F32, name="kqsc_t")
        onesF = ptile(p_const, [128, 64], F32, name="onesF")
        eps_t = ptile(p_const, [128, 1], F32, name="eps_t")
        KT = [ptile(p_att, [128, S], BF16, name=f"KT{d}") for d in range(NT_D)]
        QT = [ptile(p_att, [128, SQ], BF16, name=f"QT{d}") for d in range(NT_D)]
        x2 = [ptile(p_x2, [128, D], F32, name=f"x2_{q}") for q in range(NT_Q)]
        x2T = [ptile(p_x2, [128, SQ], BF16, name=f"x2T{d}") for d in range(NT_D)]

        nc.sync.dma_start(kqsc_t[:], kqsc[:])
        nc.vector.memset(eps_t[:], EPS)
        nc.vector.memset(onesF[:], 1.0)
        vview = Vaug[:].rearrange("p (s h k) -> p s h k", s=NT_S, h=H)
        nc.vector.memset(vview[:, :, :, HD:VW], 1.0)

        dram = top.enter_context(tc.tile_pool(name="dram", bufs=1, space="DRAM"))
        xh_d = dram.tile([S, D], BF16, name="xh_d")
        kh_d = dram.tile([S, D], BF16, name="kh_d")
        qh_d = dram.tile([SQ, D], BF16, name="qh_d")
        x2h_d = dram.tile([SQ, D], BF16, name="x2h_d")

        # =============== Phase A: ln1 + x_hat^T ===============
        p_xT = tc.alloc_tile_pool(name="p_xT", bufs=1)
        xT = [ptile(p_xT, [128, S], BF16, name=f"xT{d}") for d in range(NT_D)]
        with ExitStack() as ctx:
            io = ctx.enter_context(tc.tile_pool(name="a_io", bufs=5))
            st_p = ctx.enter_context(tc.tile_pool(name="a_stats", bufs=8))
            scr = ctx.enter_context(tc.tile_pool(name="a_scr", bufs=5))
            for t in range(NT_S):
                lt = io.tile([128, D], F32, name="lt")
                nc.sync.dma_start(lt[:], lat[t * 128:(t + 1) * 128, :])
                sq = scr.tile([128, D], F32, name="sq")
                ssq = st_p.tile([128, 1], F32, name="ssq")
                nc.scalar.activation(sq[:], lt[:], AF.Square, accum_out=ssq[:])
                srt = st_p.tile([128, 1], F32, name="srt")
                nc.scalar.activation(srt[:], ssq[:], AF.Sqrt, bias=eps_t[:], scale=1.0 / D)
                rs = st_p.tile([128, 1], F32, name="rs")
                nc.vector.reciprocal(rs[:], srt[:])
                xh = scr.tile([128, D], BF16, name="xh")
                nc.vector.tensor_scalar_mul(xh[:], lt[:], rs[:])
                nc.gpsimd.dma_start(xh_d[t * 128:(t + 1) * 128, :], xh[:])
        for d in range(NT_D):
            nc.sync.dma_start_transpose(xT[d][:], xh_d[:, d * 128:(d + 1) * 128])

        # =============== Phase B: Q/K/V projections + qk-norm ===============
        with ExitStack() as ctx:
            wp = ctx.enter_context(tc.tile_pool(name="b_w", bufs=1))
            wq_sb = [wp.tile([128, D], BF16, name=f"wq_sb{d}") for d in range(NT_D)]
            wk_sb = [wp.tile([128, D], BF16, name=f"wk_sb{d}") for d in range(NT_D)]
            wv_sb = [wp.tile([128, D], BF16, name=f"wv_sb{d}") for d in range(NT_D)]
            for d in range(NT_D):
                nc.sync.dma_start(wq_sb[d][:], wq[d * 128:(d + 1) * 128, :])
                nc.sync.dma_start(wk_sb[d][:], wk[d * 128:(d + 1) * 128, :])
                nc.sync.dma_start(wv_sb[d][:], wv[d * 128:(d + 1) * 128, :])

            ps = ctx.enter_context(tc.tile_pool(name="b_ps", bufs=3, space="PSUM"))
            scr = ctx.enter_context(tc.tile_pool(name="b_scr", bufs=3))
            st_p = ctx.enter_context(tc.tile_pool(name="b_stats", bufs=6))
            natp = ctx.enter_context(tc.tile_pool(name="b_nat", bufs=3))

            def proj(t, w_sb):
                p = ps.tile([128, D], F32, name="p_proj")
                for d in range(NT_D):
                    lhsT = xT[d][:, t * 128:(t + 1) * 128]
                    for ofs, n in _chunks(D):
                        nc.tensor.matmul(
                            p[:, ofs:ofs + n], lhsT, w_sb[d][:, ofs:ofs + n],
                            start=(d == 0), stop=(d == NT_D - 1))
                return p

            def qknorm(p, dst_dram, t):
                sq = scr.tile([128, D], F32, name="sq_b")
                nc.scalar.activation(sq[:], p[:], AF.Square)
                ss = st_p.tile([128, H], F32, name="ss_b")
                nc.vector.tensor_reduce(
                    ss[:], sq[:].rearrange("p (h k) -> p h k", h=H),
                    axis=mybir.AxisListType.X, op=OP.add)
                srt = st_p.tile([128, H], F32, name="srt_b")
                nc.scalar.activation(srt[:], ss[:], AF.Sqrt, bias=eps_t[:], scale=1.0 / HD)
                rs = st_p.tile([128, H], F32, name="rs_b")
                nc.vector.reciprocal(rs[:], srt[:])
                nat = natp.tile([128, D], BF16, name="nat_b")
                rs_view = rs[:].rearrange("p (h o) -> p h o", o=1).broadcast_to([128, H, HD])
                nc.vector.tensor_tensor(
                    out=nat[:].rearrange("p (h k) -> p h k", h=H),
                    in0=p[:].rearrange("p (h k) -> p h k", h=H),
                    in1=rs_view, op=OP.mult)
                nc.gpsimd.dma_start(dst_dram[t * 128:(t + 1) * 128, :], nat[:])

            for t in range(NT_S):
                pk = proj(t, wk_sb)
                qknorm(pk, kh_d, t)
                pv = proj(t, wv_sb)
                nc.vector.tensor_copy(
                    vview[:, t, :, 0:HD],
                    pv[:].rearrange("p (h k) -> p h k", h=H))
                if t < NT_Q:
                    pq = proj(t, wq_sb)
                    qknorm(pq, qh_d, t)
        for d in range(NT_D):
            nc.sync.dma_start_transpose(KT[d][:], kh_d[:, d * 128:(d + 1) * 128])
            nc.vector.tensor_scalar_mul(KT[d][:], KT[d][:], kqsc_t[:])
            nc.sync.dma_start_transpose(QT[d][:], qh_d[:, d * 128:(d + 1) * 128])
        p_xT.release()

        # =============== Phase C: attention ===============
        # j-split: process 512 query columns at a time so every PSUM tile is
        # one bank (psL 4 + psO 3 + psB 1 = 8 banks). Logits are emitted one
        # key-tile ahead of attn@V so ACT (the bottleneck) never starves.
        # Denominators are staged into a [H, SQ] tile via tiny SBUF->SBUF
        # DMAs (cross-partition moves), reciprocal'd 12 lanes at once, and
        # broadcast across 64 partitions with one N=512 ones-matmul per head.
        CQ = 512
        with ExitStack() as ctx:
            psL = ctx.enter_context(tc.tile_pool(name="c_psL", bufs=4, space="PSUM"))
            psO = ctx.enter_context(tc.tile_pool(name="c_psO", bufs=3, space="PSUM"))
            psB = ctx.enter_context(tc.tile_pool(name="c_psB", bufs=1, space="PSUM"))
            pp = ctx.enter_context(tc.tile_pool(name="c_p", bufs=6))
            oup = ctx.enter_context(tc.tile_pool(name="c_oU", bufs=14))
            dstp = ctx.enter_context(tc.tile_pool(name="c_dst", bufs=1))
            den = dstp.tile([H, SQ], F32, name="den")
            denr = dstp.tile([H, SQ], F32, name="denr")
            denb = dstp.tile([1, H * SQ], F32, name="denb")  # partition-0 rows

            def logits_mm(dt, t, qs, l_ps):
                for e in range(2):  # head 2*dt+e at PE row group 64*e
                    base = 64 * e
                    nc.tensor.matmul(
                        l_ps[e][:], KT[dt][base:base + 64, t * 128:(t + 1) * 128],
                        QT[dt][base:base + 64, qs], start=True, stop=True)

            for j in range(NC_Q):
                qs = slice(j * CQ, (j + 1) * CQ)
                oUs = []
                for hp in range(H // 2):
                    dt = hp
                    o_ps = [psO.tile([VW, CQ], F32, name=f"o_ps{e}", tag="o_ps")
                            for e in range(2)]
                    l_ps = [psL.tile([128, CQ], F32, name=f"l_ps{e}", tag="l_ps")
                            for e in range(2)]
                    logits_mm(dt, 0, qs, l_ps)
                    for t in range(NT_S):
                        l_nxt = None
                        if t + 1 < NT_S:
                            l_nxt = [psL.tile([128, CQ], F32, name=f"l_ps{e}",
                                              tag="l_ps") for e in range(2)]
                            logits_mm(dt, t + 1, qs, l_nxt)
                        for e in range(2):
                            p_t = pp.tile([128, CQ], BF16, name=f"p_t{e}", tag="p_t")
                            nc.scalar.activation(p_t[:], l_ps[e][:], AF.Exp)
                            h = 2 * hp + e
                            vofs = t * H * VW + h * VW
                            nc.tensor.matmul(
                                o_ps[e][:], Vaug[:, vofs:vofs + VW], p_t[:],
                                start=(t == 0), stop=(t == NT_S - 1))
                        l_ps = l_nxt
                    for e in range(2):
                        h = 2 * hp + e
                        oU = oup.tile([VW, CQ], F32, name="oU", tag="oU")
                        nc.vector.tensor_copy(oU[:], o_ps[e][:])
                        nc.sync.dma_start(den[h:h + 1, qs], oU[VW - 1:VW, :])
                        oUs.append((h, oU))
                nc.vector.reciprocal(denr[:, qs], den[:, qs])
                for h, _ in oUs:
                    nc.sync.dma_start(denb[0:1, h * SQ + j * CQ:h * SQ + (j + 1) * CQ],
                                      denr[h:h + 1, qs])
                for h, oU in oUs:
                    dt, base = h // 2, (h % 2) * 64
                    b_ps = psB.tile([64, CQ], F32, name="b_ps", tag="b_ps")
                    nc.tensor.matmul(b_ps[:], onesF[0:1, 0:64],
                                     denb[0:1, h * SQ + j * CQ:h * SQ + (j + 1) * CQ],
                                     start=True, stop=True)
                    nc.vector.scalar_tensor_tensor(
                        oT[base:base + 64, dt * SQ + j * CQ:dt * SQ + (j + 1) * CQ],
                        b_ps[:], 1.0, oU[0:HD, :], op0=OP.bypass, op1=OP.mult)
        p_att.release()

        # =============== Phase D: out-proj + residual + ln2 ===============
        with ExitStack() as ctx:
            wp = ctx.enter_context(tc.tile_pool(name="d_w", bufs=1))
            wo_sb = [wp.tile([128, D], BF16, name=f"wo_sb{d}") for d in range(NT_D)]
            for d in range(NT_D):
                nc.sync.dma_start(wo_sb[d][:], wo[d * 128:(d + 1) * 128, :])
            ps = ctx.enter_context(tc.tile_pool(name="d_ps", bufs=2, space="PSUM"))
            io = ctx.enter_context(tc.tile_pool(name="d_io", bufs=3))
            scr = ctx.enter_context(tc.tile_pool(name="d_scr", bufs=3))
            st_p = ctx.enter_context(tc.tile_pool(name="d_stats", bufs=4))

            for q in range(NT_Q):
                p = ps.tile([128, D], F32, name="p_oproj")
                for d in range(NT_D):
                    for ofs, n in _chunks(D):
                        nc.tensor.matmul(
                            p[:, ofs:ofs + n],
                            oT[:, d * SQ + q * 128: d * SQ + (q + 1) * 128],
                            wo_sb[d][:, ofs:ofs + n],
                            start=(d == 0), stop=(d == NT_D - 1))
                lt = io.tile([128, D], F32, name="lt_d")
                nc.sync.dma_start(lt[:], lat[q * 128:(q + 1) * 128, :])
                nc.vector.tensor_tensor(out=x2[q][:], in0=p[:], in1=lt[:], op=OP.add)
                sq = scr.tile([128, D], F32, name="sq_d")
                ssq = st_p.tile([128, 1], F32, name="ssq_d")
                nc.scalar.activation(sq[:], x2[q][:], AF.Square, accum_out=ssq[:])
                srt = st_p.tile([128, 1], F32, name="srt_d")
                nc.scalar.activation(srt[:], ssq[:], AF.Sqrt, bias=eps_t[:], scale=1.0 / D)
                rs = st_p.tile([128, 1], F32, name="rs_d")
                nc.vector.reciprocal(rs[:], srt[:])
                xh2 = scr.tile([128, D], BF16, name="xh2")
                nc.vector.tensor_scalar_mul(xh2[:], x2[q][:], rs[:])
                nc.gpsimd.dma_start(x2h_d[q * 128:(q + 1) * 128, :], xh2[:])
            for d in range(NT_D):
                nc.sync.dma_start_transpose(x2T[d][:], x2h_d[:, d * 128:(d + 1) * 128])
        p_oT.release()

        # =============== Phase E: MLP ===============
        p_hT = tc.alloc_tile_pool(name="p_hT", bufs=1)
        hT = ptile(p_hT, [128, NT_M * SQ], BF16, name="hT")
        with ExitStack() as ctx:
            wp = ctx.enter_context(tc.tile_pool(name="e_w", bufs=1))
            wi_sb = [wp.tile([128, MLP], BF16, name=f"wi_sb{d}") for d in range(NT_D)]
            for d in range(NT_D):
                nc.sync.dma_start(wi_sb[d][:], wi[d * 128:(d + 1) * 128, :])
            wom_sb = [wp.tile([128, D], BF16, name=f"wom_sb{m}") for m in range(NT_M)]
            for m in range(NT_M):
                nc.sync.dma_start(wom_sb[m][:], wom[m * 128:(m + 1) * 128, :])

            ps = ctx.enter_context(tc.tile_pool(name="e_ps", bufs=1, space="PSUM"))
            iop = ctx.enter_context(tc.tile_pool(name="e_io", bufs=3))

            for m in range(NT_M):
                p = ps.tile([128, SQ], F32, name="p_mlp1", bufs=2)
                for d in range(NT_D):
                    for j in range(NC_Q):
                        nc.tensor.matmul(
                            p[:, j * CH:(j + 1) * CH],
                            wi_sb[d][:, m * 128:(m + 1) * 128],
                            x2T[d][:, j * CH:(j + 1) * CH],
                            start=(d == 0), stop=(d == NT_D - 1))
                if not sim_compat:
                    nc.scalar.activation(hT[:, m * SQ:(m + 1) * SQ], p[:],
                                         AF.Gelu_apprx_tanh)
                else:
                    xsq = iop.tile([128, SQ], F32, name="g_xsq", bufs=1)
                    nc.vector.tensor_tensor(out=xsq[:], in0=p[:], in1=p[:], op=OP.mult)
                    w = iop.tile([128, SQ], F32, name="g_w", bufs=1)
                    nc.vector.tensor_scalar(w[:], xsq[:], 0.044715, 1.0,
                                            op0=OP.mult, op1=OP.add)
                    u = iop.tile([128, SQ], F32, name="g_u", bufs=1)
                    nc.vector.tensor_tensor(out=u[:], in0=w[:], in1=p[:], op=OP.mult)
                    th = iop.tile([128, SQ], F32, name="g_th", bufs=1)
                    nc.scalar.activation(th[:], u[:], AF.Tanh, scale=0.7978845608028654)
                    t2 = iop.tile([128, SQ], F32, name="g_t2", bufs=1)
                    nc.vector.scalar_tensor_tensor(t2[:], th[:], 1.0, p[:],
                                                   op0=OP.add, op1=OP.mult)
                    nc.vector.tensor_scalar_mul(hT[:, m * SQ:(m + 1) * SQ], t2[:], 0.5)

            for q in range(NT_Q):
                p = ps.tile([128, D], F32, name="p_mlp2", bufs=2)
                for m in range(NT_M):
                    for ofs, n in _chunks(D):
                        nc.tensor.matmul(
                            p[:, ofs:ofs + n],
                            hT[:, m * SQ + q * 128: m * SQ + (q + 1) * 128],
                            wom_sb[m][:, ofs:ofs + n],
                            start=(m == 0), stop=(m == NT_M - 1))
                ot = iop.tile([128, D], F32, name="ot_e")
                nc.vector.tensor_tensor(out=ot[:], in0=p[:], in1=x2[q][:], op=OP.add)
                nc.sync.dma_start(out[q * 128:(q + 1) * 128, :], ot[:])
        p_hT.release()

    nc.compile()
    return nc


def make_in_maps(latents, ln1_scale, wq, wk, wv, q_norm_scale, k_norm_scale,
                 wo_attn, ln2_scale, wi, wo_mlp):
    import ml_dtypes
    bf = ml_dtypes.bfloat16
    wq2 = (np.asarray(ln1_scale, np.float64)[:, None]
           * np.asarray(wq, np.float64).reshape(D, D)).astype(bf)
    wk2 = (np.asarray(ln1_scale, np.float64)[:, None]
           * np.asarray(wk, np.float64).reshape(D, D)).astype(bf)
    wv2 = (np.asarray(ln1_scale, np.float64)[:, None]
           * np.asarray(wv, np.float64).reshape(D, D)).astype(bf)
    wo2 = np.asarray(wo_attn, np.float32).reshape(D, D).astype(bf)
    wi2 = (np.asarray(ln2_scale, np.float64)[:, None]
           * np.asarray(wi, np.float64)).astype(bf)
    wom2 = np.asarray(wo_mlp, np.float32).astype(bf)
    kq = (np.tile(np.asarray(q_norm_scale, np.float64)
                  * np.asarray(k_norm_scale, np.float64), 2)
          / np.sqrt(HD)).astype(np.float32)[:, None]
    lat_np = np.asarray(latents, np.float32)
    in_maps = []
    for c in range(8):
        b, half = c // 2, c % 2
        lm = lat_np[b]
        lat_rot = np.concatenate([lm[half * SQ:(half + 1) * SQ],
                                  lm[(1 - half) * SQ:(2 - half) * SQ]], axis=0)
        in_maps.append(dict(lat=np.ascontiguousarray(lat_rot), wq=wq2, wk=wk2,
                            wv=wv2, wo=wo2, wi=wi2, wom=wom2, kqsc=kq))
    return in_maps


_NC_CACHE = None


def kernel(**inputs):
    global _NC_CACHE
    if _NC_CACHE is None:
        _NC_CACHE = build_nc()
    nc = _NC_CACHE
    in_maps = make_in_maps(**inputs)
    res = run_bass_kernel_spmd(nc, in_maps, list(range(8)))
    y = np.empty((B, S, D), np.float32)
    for c in range(8):
        b, half = c // 2, c % 2
        y[b, half * SQ:(half + 1) * SQ] = res.results[c]["out"]
    return y


if __name__ == "__main__":
    import reference
    inputs = {k: np.asarray(v) for k, v in reference.setup_inputs().items()}
    y = kernel(**inputs)
    exp = np.asarray(reference.reference(**reference.setup_inputs()))
    err = np.abs(y - exp).max() / np.abs(exp).max()
    print("Relative error:", err)



# revision 7
# speedup vs baseline: 1.2554x; 1.1266x over previous
"""Fused transformer block (RMSNorm + qk-norm attention + MLP) for TRN2, 8 cores.

Sharding: 8 cores = (4 batches) x (2 query-halves). Each core gets its batch's
full sequence with rows rotated so its query half is rows 0..1023 (attention is
permutation-invariant over keys, so K/V row order doesn't matter). No
collectives needed; each core produces a disjoint [1024, 768] output slice.

Layout strategy per core:
  - x_hat = rmsnorm(lat) in natural [s, d] layout, cast bf16, round-tripped
    through DRAM with chunked DMA-transposes (512-row chunks pipeline with
    producers/consumers) to get x_hat^T [d, s] for the projections.
  - Q/K projections in natural layout (lhsT = x_hat^T tile), qk-rmsnorm applied
    in natural layout, then DMA-transposed to Q^T/K^T [hd, s] per head pair.
  - logits^T[k, q] = K^T_h.T @ Q^T_h; head pairs issue to disjoint PE row
    groups (partition base 0/64) so both heads' logits matmuls overlap.
    exp on ScalarE with no max subtraction (|logit| <= 8: q, k are unit-RMS).
    P[k, q] feeds attn@V with V in natural [k, hd] layout augmented with a
    ones column -> softmax denominator lands in PSUM row 64 for free.
  - softmax division is decoupled from the PE stream: accumulators are
    evacuated to SBUF, reciprocals run on VectorE under the ACT-bound inner
    loop, and the ones-outer-product broadcasts + multiplies run at phase end.
  - out-proj / MLP2 use the activation tile as the stationary operand so the
    result comes out in natural [q, d] layout for residuals.
"""

import numpy as np
from contextlib import ExitStack

import concourse.bass as bass
import concourse.tile as tile
from concourse import bacc, mybir
from concourse.bass_utils import run_bass_kernel_spmd

F32 = mybir.dt.float32
BF16 = mybir.dt.bfloat16
AF = mybir.ActivationFunctionType
OP = mybir.AluOpType

B, S, D, H, HD, MLP = 4, 2048, 768, 12, 64, 3072
SQ = S // 2            # query rows per core
NT_S = S // 128        # 16 sequence tiles
NT_Q = SQ // 128       # 8 query tiles
NT_D = D // 128        # 6 model-dim tiles
NT_M = MLP // 128      # 24 mlp-dim tiles
EPS = 1e-6
VW = HD + 1            # V width incl. ones column
CH = 512               # transpose chunk (rows)
NC_S = S // CH         # 4 chunks over full sequence
NC_Q = SQ // CH        # 2 chunks over query rows


def _chunks(n):
    out, ofs = [], 0
    while ofs < n:
        c = min(512, n - ofs)
        out.append((ofs, c))
        ofs += c
    return out


def build_nc(sim_compat=False):
    nc = bacc.Bacc("TRN2", target_bir_lowering=False, debug=False, num_devices=8)

    lat = nc.dram_tensor("lat", [S, D], F32, kind="ExternalInput").ap()
    wq = nc.dram_tensor("wq", [D, D], BF16, kind="ExternalInput").ap()
    wk = nc.dram_tensor("wk", [D, D], BF16, kind="ExternalInput").ap()
    wv = nc.dram_tensor("wv", [D, D], BF16, kind="ExternalInput").ap()
    wo = nc.dram_tensor("wo", [D, D], BF16, kind="ExternalInput").ap()
    wi = nc.dram_tensor("wi", [D, MLP], BF16, kind="ExternalInput").ap()
    wom = nc.dram_tensor("wom", [MLP, D], BF16, kind="ExternalInput").ap()
    kqsc = nc.dram_tensor("kqsc", [128, 1], F32, kind="ExternalInput").ap()
    out = nc.dram_tensor("out", [SQ, D], F32, kind="ExternalOutput").ap()

    with tile.TileContext(nc) as tc, ExitStack() as top:
        def ptile(pool, shape, dtype, name):
            return pool.tile(shape, dtype, name=name, tag=name)

        p_const = top.enter_context(tc.tile_pool(name="p_const", bufs=1))
        p_x2 = top.enter_context(tc.tile_pool(name="p_x2", bufs=1))
        p_oT = tc.alloc_tile_pool(name="p_oT", bufs=1)
        p_att = tc.alloc_tile_pool(name="p_att", bufs=1)

        # ---- persistent tiles ----
        Vaug = ptile(p_att, [128, NT_S * H * VW], BF16, name="Vaug")
        oT = ptile(p_oT, [128, NT_D * SQ], BF16, name="oT")
        kqsc_t = ptile(p_const, [128, 1], F32, name="kqsc_t")
        onesF = ptile(p_const, [128, 64], F32, name="onesF")
        eps_t = ptile(p_const, [128, 1], F32, name="eps_t")
        KT = [ptile(p_att, [128, S], BF16, name=f"KT{d}") for d in range(NT_D)]
        QT = [ptile(p_att, [128, SQ], BF16, name=f"QT{d}") for d in range(NT_D)]
        x2 = [ptile(p_x2, [128, D], F32, name=f"x2_{q}") for q in range(NT_Q)]
        x2T = [ptile(p_x2, [128, SQ], BF16, name=f"x2T{d}") for d in range(NT_D)]

        nc.sync.dma_start(kqsc_t[:], kqsc[:])
        nc.vector.memset(eps_t[:], EPS)
        nc.vector.memset(onesF[:], 1.0)
        vview = Vaug[:].rearrange("p (s h k) -> p s h k", s=NT_S, h=H)
        nc.vector.memset(vview[:, :, :, HD:VW], 1.0)

        dram = top.enter_context(tc.tile_pool(name="dram", bufs=1, space="DRAM"))
        xh_d = dram.tile([S, D], BF16, name="xh_d")
        kh_d = dram.tile([S, D], BF16, name="kh_d")
        qh_d = dram.tile([SQ, D], BF16, name="qh_d")
        x2h_d = dram.tile([SQ, D], BF16, name="x2h_d")

        # =============== Phase A: ln1 + x_hat^T ===============
        p_xT = tc.alloc_tile_pool(name="p_xT", bufs=1)
        xT = [ptile(p_xT, [128, S], BF16, name=f"xT{d}") for d in range(NT_D)]
        with ExitStack() as ctx:
            io = ctx.enter_context(tc.tile_pool(name="a_io", bufs=5))
            st_p = ctx.enter_context(tc.tile_pool(name="a_stats", bufs=8))
            scr = ctx.enter_context(tc.tile_pool(name="a_scr", bufs=5))
            for t in range(NT_S):
                lt = io.tile([128, D], F32, name="lt")
                nc.sync.dma_start(lt[:], lat[t * 128:(t + 1) * 128, :])
                sq = scr.tile([128, D], F32, name="sq")
                ssq = st_p.tile([128, 1], F32, name="ssq")
                nc.scalar.activation(sq[:], lt[:], AF.Square, accum_out=ssq[:])
                srt = st_p.tile([128, 1], F32, name="srt")
                nc.scalar.activation(srt[:], ssq[:], AF.Sqrt, bias=eps_t[:], scale=1.0 / D)
                rs = st_p.tile([128, 1], F32, name="rs")
                nc.vector.reciprocal(rs[:], srt[:])
                xh = scr.tile([128, D], BF16, name="xh")
                nc.vector.tensor_scalar_mul(xh[:], lt[:], rs[:])
                nc.gpsimd.dma_start(xh_d[t * 128:(t + 1) * 128, :], xh[:])
        for d in range(NT_D):
            nc.sync.dma_start_transpose(xT[d][:], xh_d[:, d * 128:(d + 1) * 128])

        # =============== Phase B: Q/K/V projections + qk-norm ===============
        with ExitStack() as ctx:
            wp = ctx.enter_context(tc.tile_pool(name="b_w", bufs=1))
            wq_sb = [wp.tile([128, D], BF16, name=f"wq_sb{d}") for d in range(NT_D)]
            wk_sb = [wp.tile([128, D], BF16, name=f"wk_sb{d}") for d in range(NT_D)]
            wv_sb = [wp.tile([128, D], BF16, name=f"wv_sb{d}") for d in range(NT_D)]
            for d in range(NT_D):
                nc.sync.dma_start(wq_sb[d][:], wq[d * 128:(d + 1) * 128, :])
                nc.sync.dma_start(wk_sb[d][:], wk[d * 128:(d + 1) * 128, :])
                nc.sync.dma_start(wv_sb[d][:], wv[d * 128:(d + 1) * 128, :])

            ps = ctx.enter_context(tc.tile_pool(name="b_ps", bufs=3, space="PSUM"))
            scr = ctx.enter_context(tc.tile_pool(name="b_scr", bufs=3))
            st_p = ctx.enter_context(tc.tile_pool(name="b_stats", bufs=6))
            natp = ctx.enter_context(tc.tile_pool(name="b_nat", bufs=3))

            def proj(t, w_sb):
                p = ps.tile([128, D], F32, name="p_proj")
                for d in range(NT_D):
                    lhsT = xT[d][:, t * 128:(t + 1) * 128]
                    for ofs, n in _chunks(D):
                        nc.tensor.matmul(
                            p[:, ofs:ofs + n], lhsT, w_sb[d][:, ofs:ofs + n],
                            start=(d == 0), stop=(d == NT_D - 1))
                return p

            def qknorm(p, dst_dram, t):
                sq = scr.tile([128, D], F32, name="sq_b")
                nc.scalar.activation(sq[:], p[:], AF.Square)
                ss = st_p.tile([128, H], F32, name="ss_b")
                nc.vector.tensor_reduce(
                    ss[:], sq[:].rearrange("p (h k) -> p h k", h=H),
                    axis=mybir.AxisListType.X, op=OP.add)
                srt = st_p.tile([128, H], F32, name="srt_b")
                nc.scalar.activation(srt[:], ss[:], AF.Sqrt, bias=eps_t[:], scale=1.0 / HD)
                rs = st_p.tile([128, H], F32, name="rs_b")
                nc.vector.reciprocal(rs[:], srt[:])
                nat = natp.tile([128, D], BF16, name="nat_b")
                rs_view = rs[:].rearrange("p (h o) -> p h o", o=1).broadcast_to([128, H, HD])
                nc.vector.tensor_tensor(
                    out=nat[:].rearrange("p (h k) -> p h k", h=H),
                    in0=p[:].rearrange("p (h k) -> p h k", h=H),
                    in1=rs_view, op=OP.mult)
                nc.gpsimd.dma_start(dst_dram[t * 128:(t + 1) * 128, :], nat[:])

            for t in range(NT_S):
                pk = proj(t, wk_sb)
                qknorm(pk, kh_d, t)
                pv = proj(t, wv_sb)
                nc.vector.tensor_copy(
                    vview[:, t, :, 0:HD],
                    pv[:].rearrange("p (h k) -> p h k", h=H))
                if t < NT_Q:
                    pq = proj(t, wq_sb)
                    qknorm(pq, qh_d, t)
        for d in range(NT_D):
            nc.sync.dma_start_transpose(KT[d][:], kh_d[:, d * 128:(d + 1) * 128])
            nc.vector.tensor_scalar_mul(KT[d][:], KT[d][:], kqsc_t[:])
            nc.sync.dma_start_transpose(QT[d][:], qh_d[:, d * 128:(d + 1) * 128])
        p_xT.release()

        # =============== Phase C: attention ===============
        # j-split: process 512 query columns at a time so every PSUM tile is
        # one bank (psL 4 + psO 3 + psB 1 = 8 banks). Logits are emitted one
        # key-tile ahead of attn@V so ACT (the bottleneck) never starves.
        # Denominators are staged into a [H, SQ] tile via tiny SBUF->SBUF
        # DMAs (cross-partition moves), reciprocal'd 12 lanes at once, and
        # broadcast across 64 partitions with one N=512 ones-matmul per head.
        CQ = 512
        with ExitStack() as ctx:
            psL = ctx.enter_context(tc.tile_pool(name="c_psL", bufs=2, space="PSUM"))
            psO = ctx.enter_context(tc.tile_pool(name="c_psO", bufs=3, space="PSUM"))
            psB = ctx.enter_context(tc.tile_pool(name="c_psB", bufs=1, space="PSUM"))
            pp = ctx.enter_context(tc.tile_pool(name="c_p", bufs=4))
            oup = ctx.enter_context(tc.tile_pool(name="c_oU", bufs=14))
            dstp = ctx.enter_context(tc.tile_pool(name="c_dst", bufs=1))
            den = dstp.tile([H, SQ], F32, name="den")
            denr = dstp.tile([H, SQ], F32, name="denr")
            denb = dstp.tile([1, H * SQ], F32, name="denb")  # partition-0 rows

            def logits_mm(dt, t, qs, l_ps):
                for e in range(2):  # head 2*dt+e at PE row group 64*e
                    base = 64 * e
                    nc.tensor.matmul(
                        l_ps[:, e * CQ:(e + 1) * CQ],
                        KT[dt][base:base + 64, t * 128:(t + 1) * 128],
                        QT[dt][base:base + 64, qs], start=True, stop=True)

            for j in range(NC_Q):
                qs = slice(j * CQ, (j + 1) * CQ)
                oUs = []
                for hp in range(H // 2):
                    dt = hp
                    o_ps = [psO.tile([VW, CQ], F32, name=f"o_ps{e}", tag="o_ps")
                            for e in range(2)]
                    l_ps = psL.tile([128, 2 * CQ], F32, name="l_ps", tag="l_ps")
                    logits_mm(dt, 0, qs, l_ps)
                    for t in range(NT_S):
                        l_nxt = None
                        if t + 1 < NT_S:
                            l_nxt = psL.tile([128, 2 * CQ], F32, name="l_ps",
                                             tag="l_ps")
                            logits_mm(dt, t + 1, qs, l_nxt)
                        p_t = pp.tile([128, 2 * CQ], BF16, name="p_t", tag="p_t")
                        nc.scalar.activation(p_t[:], l_ps[:], AF.Exp)
                        for e in range(2):
                            h = 2 * hp + e
                            vofs = t * H * VW + h * VW
                            nc.tensor.matmul(
                                o_ps[e][:], Vaug[:, vofs:vofs + VW],
                                p_t[:, e * CQ:(e + 1) * CQ],
                                start=(t == 0), stop=(t == NT_S - 1))
                        l_ps = l_nxt
                    for e in range(2):
                        h = 2 * hp + e
                        oU = oup.tile([VW, CQ], F32, name="oU", tag="oU")
                        nc.vector.tensor_copy(oU[:], o_ps[e][:])
                        nc.sync.dma_start(den[h:h + 1, qs], oU[VW - 1:VW, :])
                        oUs.append((h, oU))
                nc.vector.reciprocal(denr[:, qs], den[:, qs])
                for h, _ in oUs:
                    nc.sync.dma_start(denb[0:1, h * SQ + j * CQ:h * SQ + (j + 1) * CQ],
                                      denr[h:h + 1, qs])
                for h, oU in oUs:
                    dt, base = h // 2, (h % 2) * 64
                    b_ps = psB.tile([64, CQ], F32, name="b_ps", tag="b_ps")
                    nc.tensor.matmul(b_ps[:], onesF[0:1, 0:64],
                                     denb[0:1, h * SQ + j * CQ:h * SQ + (j + 1) * CQ],
                                     start=True, stop=True)
                    nc.vector.scalar_tensor_tensor(
                        oT[base:base + 64, dt * SQ + j * CQ:dt * SQ + (j + 1) * CQ],
                        b_ps[:], 1.0, oU[0:HD, :], op0=OP.bypass, op1=OP.mult)
        p_att.release()

        # =============== Phase D: out-proj + residual + ln2 ===============
        with ExitStack() as ctx:
            wp = ctx.enter_context(tc.tile_pool(name="d_w", bufs=1))
            wo_sb = [wp.tile([128, D], BF16, name=f"wo_sb{d}") for d in range(NT_D)]
            for d in range(NT_D):
                nc.sync.dma_start(wo_sb[d][:], wo[d * 128:(d + 1) * 128, :])
            ps = ctx.enter_context(tc.tile_pool(name="d_ps", bufs=2, space="PSUM"))
            io = ctx.enter_context(tc.tile_pool(name="d_io", bufs=3))
            scr = ctx.enter_context(tc.tile_pool(name="d_scr", bufs=3))
            st_p = ctx.enter_context(tc.tile_pool(name="d_stats", bufs=4))

            for q in range(NT_Q):
                p = ps.tile([128, D], F32, name="p_oproj")
                for d in range(NT_D):
                    for ofs, n in _chunks(D):
                        nc.tensor.matmul(
                            p[:, ofs:ofs + n],
                            oT[:, d * SQ + q * 128: d * SQ + (q + 1) * 128],
                            wo_sb[d][:, ofs:ofs + n],
                            start=(d == 0), stop=(d == NT_D - 1))
                lt = io.tile([128, D], F32, name="lt_d")
                nc.sync.dma_start(lt[:], lat[q * 128:(q + 1) * 128, :])
                nc.vector.tensor_tensor(out=x2[q][:], in0=p[:], in1=lt[:], op=OP.add)
                sq = scr.tile([128, D], F32, name="sq_d")
                ssq = st_p.tile([128, 1], F32, name="ssq_d")
                nc.scalar.activation(sq[:], x2[q][:], AF.Square, accum_out=ssq[:])
                srt = st_p.tile([128, 1], F32, name="srt_d")
                nc.scalar.activation(srt[:], ssq[:], AF.Sqrt, bias=eps_t[:], scale=1.0 / D)
                rs = st_p.tile([128, 1], F32, name="rs_d")
                nc.vector.reciprocal(rs[:], srt[:])
                xh2 = scr.tile([128, D], BF16, name="xh2")
                nc.vector.tensor_scalar_mul(xh2[:], x2[q][:], rs[:])
                nc.gpsimd.dma_start(x2h_d[q * 128:(q + 1) * 128, :], xh2[:])
            for d in range(NT_D):
                nc.sync.dma_start_transpose(x2T[d][:], x2h_d[:, d * 128:(d + 1) * 128])
        p_oT.release()

        # =============== Phase E: MLP ===============
        p_hT = tc.alloc_tile_pool(name="p_hT", bufs=1)
        hT = ptile(p_hT, [128, NT_M * SQ], BF16, name="hT")
        with ExitStack() as ctx:
            wp = ctx.enter_context(tc.tile_pool(name="e_w", bufs=1))
            wi_sb = [wp.tile([128, MLP], BF16, name=f"wi_sb{d}") for d in range(NT_D)]
            for d in range(NT_D):
                nc.sync.dma_start(wi_sb[d][:], wi[d * 128:(d + 1) * 128, :])
            wom_sb = [wp.tile([128, D], BF16, name=f"wom_sb{m}") for m in range(NT_M)]
            for m in range(NT_M):
                nc.sync.dma_start(wom_sb[m][:], wom[m * 128:(m + 1) * 128, :])

            ps = ctx.enter_context(tc.tile_pool(name="e_ps", bufs=1, space="PSUM"))
            iop = ctx.enter_context(tc.tile_pool(name="e_io", bufs=3))

            for m in range(NT_M):
                p = ps.tile([128, SQ], F32, name="p_mlp1", bufs=2)
                for d in range(NT_D):
                    for j in range(NC_Q):
                        nc.tensor.matmul(
                            p[:, j * CH:(j + 1) * CH],
                            wi_sb[d][:, m * 128:(m + 1) * 128],
                            x2T[d][:, j * CH:(j + 1) * CH],
                            start=(d == 0), stop=(d == NT_D - 1))
                if not sim_compat:
                    nc.scalar.activation(hT[:, m * SQ:(m + 1) * SQ], p[:],
                                         AF.Gelu_apprx_tanh)
                else:
                    xsq = iop.tile([128, SQ], F32, name="g_xsq", bufs=1)
                    nc.vector.tensor_tensor(out=xsq[:], in0=p[:], in1=p[:], op=OP.mult)
                    w = iop.tile([128, SQ], F32, name="g_w", bufs=1)
                    nc.vector.tensor_scalar(w[:], xsq[:], 0.044715, 1.0,
                                            op0=OP.mult, op1=OP.add)
                    u = iop.tile([128, SQ], F32, name="g_u", bufs=1)
                    nc.vector.tensor_tensor(out=u[:], in0=w[:], in1=p[:], op=OP.mult)
                    th = iop.tile([128, SQ], F32, name="g_th", bufs=1)
                    nc.scalar.activation(th[:], u[:], AF.Tanh, scale=0.7978845608028654)
                    t2 = iop.tile([128, SQ], F32, name="g_t2", bufs=1)
                    nc.vector.scalar_tensor_tensor(t2[:], th[:], 1.0, p[:],
                                                   op0=OP.add, op1=OP.mult)
                    nc.vector.tensor_scalar_mul(hT[:, m * SQ:(m + 1) * SQ], t2[:], 0.5)

            for q in range(NT_Q):
                p = ps.tile([128, D], F32, name="p_mlp2", bufs=2)
                for m in range(NT_M):
                    for ofs, n in _chunks(D):
                        nc.tensor.matmul(
                            p[:, ofs:ofs + n],
                            hT[:, m * SQ + q * 128: m * SQ + (q + 1) * 128],
                            wom_sb[m][:, ofs:ofs + n],
                            start=(m == 0), stop=(m == NT_M - 1))
                ot = iop.tile([128, D], F32, name="ot_e")
                nc.vector.tensor_tensor(out=ot[:], in0=p[:], in1=x2[q][:], op=OP.add)
                nc.sync.dma_start(out[q * 128:(q + 1) * 128, :], ot[:])
        p_hT.release()

    nc.compile()
    return nc


def make_in_maps(latents, ln1_scale, wq, wk, wv, q_norm_scale, k_norm_scale,
                 wo_attn, ln2_scale, wi, wo_mlp):
    import ml_dtypes
    bf = ml_dtypes.bfloat16
    wq2 = (np.asarray(ln1_scale, np.float64)[:, None]
           * np.asarray(wq, np.float64).reshape(D, D)).astype(bf)
    wk2 = (np.asarray(ln1_scale, np.float64)[:, None]
           * np.asarray(wk, np.float64).reshape(D, D)).astype(bf)
    wv2 = (np.asarray(ln1_scale, np.float64)[:, None]
           * np.asarray(wv, np.float64).reshape(D, D)).astype(bf)
    wo2 = np.asarray(wo_attn, np.float32).reshape(D, D).astype(bf)
    wi2 = (np.asarray(ln2_scale, np.float64)[:, None]
           * np.asarray(wi, np.float64)).astype(bf)
    wom2 = np.asarray(wo_mlp, np.float32).astype(bf)
    kq = (np.tile(np.asarray(q_norm_scale, np.float64)
                  * np.asarray(k_norm_scale, np.float64), 2)
          / np.sqrt(HD)).astype(np.float32)[:, None]
    lat_np = np.asarray(latents, np.float32)
    in_maps = []
    for c in range(8):
        b, half = c // 2, c % 2
        lm = lat_np[b]
        lat_rot = np.concatenate([lm[half * SQ:(half + 1) * SQ],
                                  lm[(1 - half) * SQ:(2 - half) * SQ]], axis=0)
        in_maps.append(dict(lat=np.ascontiguousarray(lat_rot), wq=wq2, wk=wk2,
                            wv=wv2, wo=wo2, wi=wi2, wom=wom2, kqsc=kq))
    return in_maps


_NC_CACHE = None


def kernel(**inputs):
    global _NC_CACHE
    if _NC_CACHE is None:
        _NC_CACHE = build_nc()
    nc = _NC_CACHE
    in_maps = make_in_maps(**inputs)
    res = run_bass_kernel_spmd(nc, in_maps, list(range(8)))
    y = np.empty((B, S, D), np.float32)
    for c in range(8):
        b, half = c // 2, c % 2
        y[b, half * SQ:(half + 1) * SQ] = res.results[c]["out"]
    return y


if __name__ == "__main__":
    import reference
    inputs = {k: np.asarray(v) for k, v in reference.setup_inputs().items()}
    y = kernel(**inputs)
    exp = np.asarray(reference.reference(**reference.setup_inputs()))
    err = np.abs(y - exp).max() / np.abs(exp).max()
    print("Relative error:", err)



# revision 16
# speedup vs baseline: 1.3632x; 1.0858x over previous
"""Fused transformer block (RMSNorm + qk-norm attention + MLP) for TRN2, 8 cores.

Sharding: 8 cores = (4 batches) x (2 query-halves). Each core gets its batch's
full sequence with rows rotated so its query half is rows 0..1023 (attention is
permutation-invariant over keys, so K/V row order doesn't matter). No
collectives needed; each core produces a disjoint [1024, 768] output slice.

Layout strategy per core:
  - Phase AB (fused): per 128-row tile, ln1 on ScalarE (Square+accum, Sqrt,
    Copy-with-per-partition-scale; only the reciprocal is on VectorE), then a
    single SBUF->SBUF DMA-transpose ([128,768] -> six d-major column blocks)
    feeds the Q/K/V projections for that tile immediately. qk-rmsnorm in
    natural layout, transposed the same way into KT/QT [hd, s]. No DRAM
    round-trips for intermediates.
  - The qk-norm scales and 1/sqrt(HD) fold into the EXP's free scale operand
    when q_norm_scale*k_norm_scale is a constant vector (it is here); the
    general fallback multiplies per-channel columns into KT post-transpose.
  - Phase C: per 512-query chunk j, per head-pair: both heads' logits land in
    one [128,1024] PSUM tile (two N=512 matmuls to PE row groups 0/64), ONE
    Exp on ScalarE covers both heads (ACT is the bottleneck engine; this
    halves call overhead), then two attn@V matmuls accumulate o^T [65,512]
    with a ones-column on V providing the softmax denominator for free.
    Logits are emitted one key-tile ahead so ACT never starves.
  - Denominator rows (one partition each) are staged via SBUF->SBUF DMA into
    a [12, SQ] tile, reciprocal'd across 12 lanes at once, cast to bf16 on a
    partition-0 row (SWDGE cast-DMA), broadcast across 64 partitions with one
    bf16 K=1 ones-matmul per head, and folded into the oT evacuation multiply.
  - wo/wi weight loads are emitted before phase C (DMA idles there), wom at
    phase D, so the MLP never waits on HBM.
  - Phase D: out-proj with the oT slice stationary -> natural [q, d] + residual
    + ln2; x2^T produced per-tile by the same SBUF->SBUF transpose.
  - Phase E: MLP1 per m-tile -> Gelu -> hT; MLP2 with hT stationary + residual.
"""

import numpy as np
from contextlib import ExitStack

import concourse.bass as bass
import concourse.tile as tile
from concourse import bacc, mybir
from concourse.bass_utils import run_bass_kernel_spmd

F32 = mybir.dt.float32
BF16 = mybir.dt.bfloat16
AF = mybir.ActivationFunctionType
OP = mybir.AluOpType

B, S, D, H, HD, MLP = 4, 2048, 768, 12, 64, 3072
SQ = S // 2            # query rows per core
NT_S = S // 128        # 16 sequence tiles
NT_Q = SQ // 128       # 8 query tiles
NT_D = D // 128        # 6 model-dim tiles
NT_M = MLP // 128      # 24 mlp-dim tiles
EPS = 1e-6
VW = HD + 1            # V width incl. ones column
CQ = 512               # attention query chunk


def _chunks(n):
    out, ofs = [], 0
    while ofs < n:
        c = min(512, n - ofs)
        out.append((ofs, c))
        ofs += c
    return out


def build_nc(exp_scale=None, sim_compat=False):
    nc = bacc.Bacc("TRN2", target_bir_lowering=False, debug=False, num_devices=8)

    lat = nc.dram_tensor("lat", [S, D], F32, kind="ExternalInput").ap()
    wq = nc.dram_tensor("wq", [D, D], BF16, kind="ExternalInput").ap()
    wk = nc.dram_tensor("wk", [D, D], BF16, kind="ExternalInput").ap()
    wv = nc.dram_tensor("wv", [D, D], BF16, kind="ExternalInput").ap()
    wo = nc.dram_tensor("wo", [D, D], BF16, kind="ExternalInput").ap()
    wi = nc.dram_tensor("wi", [D, MLP], BF16, kind="ExternalInput").ap()
    wom = nc.dram_tensor("wom", [MLP, D], BF16, kind="ExternalInput").ap()
    kqc = nc.dram_tensor("kqc", [128, NT_D], F32, kind="ExternalInput").ap()
    out = nc.dram_tensor("out", [SQ, D], F32, kind="ExternalOutput").ap()

    with tile.TileContext(nc) as tc, ExitStack() as top:
        def ptile(pool, shape, dtype, name):
            return pool.tile(shape, dtype, name=name, tag=name)

        p_const = top.enter_context(tc.tile_pool(name="p_const", bufs=1))
        p_oT = top.enter_context(tc.tile_pool(name="p_oT", bufs=1))
        p_wo = top.enter_context(tc.tile_pool(name="p_wo", bufs=1))
        p_wi = top.enter_context(tc.tile_pool(name="p_wi", bufs=1))
        p_att = tc.alloc_tile_pool(name="p_att", bufs=1)

        # ---- persistent tiles ----
        Vaug = ptile(p_att, [128, NT_S * H * VW], BF16, name="Vaug")
        KTb = ptile(p_att, [128, NT_D * S], BF16, name="KTb")
        QTb = ptile(p_att, [128, NT_D * SQ], BF16, name="QTb")
        oT = ptile(p_oT, [128, NT_D * SQ], BF16, name="oT")
        eps_t = ptile(p_const, [128, 1], F32, name="eps_t")
        onesB = ptile(p_const, [1, 64], BF16, name="onesB")
        kqc_t = ptile(p_const, [128, NT_D], F32, name="kqc_t")

        nc.vector.memset(eps_t[:], EPS)
        nc.vector.memset(onesB[:], 1.0)
        if exp_scale is None:
            nc.sync.dma_start(kqc_t[:], kqc[:])
        vview = Vaug[:].rearrange("p (s h k) -> p s h k", s=NT_S, h=H)
        nc.vector.memset(vview[:, :, :, HD:VW], 1.0)
        KTv = KTb[:].rearrange("p (d s) -> p d s", d=NT_D)
        QTv = QTb[:].rearrange("p (d s) -> p d s", d=NT_D)

        # =============== Phase AB: ln1 + projections + transposes ===========
        p_xT = tc.alloc_tile_pool(name="p_xT", bufs=1)
        xTb = ptile(p_xT, [128, NT_D * S], BF16, name="xTb")
        xTv = xTb[:].rearrange("p (d s) -> p d s", d=NT_D)
        with ExitStack() as ctx:
            wp = ctx.enter_context(tc.tile_pool(name="ab_w", bufs=1))
            wq_sb = [wp.tile([128, D], BF16, name=f"wq_sb{d}") for d in range(NT_D)]
            wk_sb = [wp.tile([128, D], BF16, name=f"wk_sb{d}") for d in range(NT_D)]
            wv_sb = [wp.tile([128, D], BF16, name=f"wv_sb{d}") for d in range(NT_D)]
            for d in range(NT_D):
                nc.sync.dma_start(wq_sb[d][:], wq[d * 128:(d + 1) * 128, :])
                nc.sync.dma_start(wk_sb[d][:], wk[d * 128:(d + 1) * 128, :])
                nc.sync.dma_start(wv_sb[d][:], wv[d * 128:(d + 1) * 128, :])

            io = ctx.enter_context(tc.tile_pool(name="ab_io", bufs=3))
            ps = ctx.enter_context(tc.tile_pool(name="ab_ps", bufs=3, space="PSUM"))
            scr = ctx.enter_context(tc.tile_pool(name="ab_scr", bufs=4))
            st_p = ctx.enter_context(tc.tile_pool(name="ab_st", bufs=10))
            natp = ctx.enter_context(tc.tile_pool(name="ab_nat", bufs=4))

            def proj(t, w_sb):
                p = ps.tile([128, D], F32, name="p_proj", tag="p_proj")
                for d in range(NT_D):
                    lhsT = xTb[:, d * S + t * 128:d * S + (t + 1) * 128]
                    for ofs, n in _chunks(D):
                        nc.tensor.matmul(
                            p[:, ofs:ofs + n], lhsT, w_sb[d][:, ofs:ofs + n],
                            start=(d == 0), stop=(d == NT_D - 1))
                return p

            def qknorm(p):
                sq = scr.tile([128, D], F32, name="sq_b", tag="sq")
                nc.scalar.activation(sq[:], p[:], AF.Square)
                ss = st_p.tile([128, H], F32, name="ss_b", tag="ss_b")
                nc.vector.tensor_reduce(
                    ss[:], sq[:].rearrange("p (h k) -> p h k", h=H),
                    axis=mybir.AxisListType.X, op=OP.add)
                srt = st_p.tile([128, H], F32, name="srt_b", tag="srt_b")
                nc.scalar.activation(srt[:], ss[:], AF.Sqrt, bias=eps_t[:],
                                     scale=1.0 / HD)
                rs = st_p.tile([128, H], F32, name="rs_b", tag="rs_b")
                nc.vector.reciprocal(rs[:], srt[:])
                nat = natp.tile([128, D], BF16, name="nat_b", tag="nat")
                rs_view = rs[:].rearrange("p (h o) -> p h o", o=1) \
                               .broadcast_to([128, H, HD])
                nc.vector.tensor_tensor(
                    out=nat[:].rearrange("p (h k) -> p h k", h=H),
                    in0=p[:].rearrange("p (h k) -> p h k", h=H),
                    in1=rs_view, op=OP.mult)
                return nat

            for t in range(NT_S):
                lt = io.tile([128, D], F32, name="lt", tag="lt")
                nc.sync.dma_start(lt[:], lat[t * 128:(t + 1) * 128, :])
                sq = scr.tile([128, D], F32, name="sq_a", tag="sq")
                ssq = st_p.tile([128, 1], F32, name="ssq", tag="ssq")
                nc.scalar.activation(sq[:], lt[:], AF.Square, accum_out=ssq[:])
                srt = st_p.tile([128, 1], F32, name="srt", tag="srt")
                nc.scalar.activation(srt[:], ssq[:], AF.Sqrt, bias=eps_t[:],
                                     scale=1.0 / D)
                rs = st_p.tile([128, 1], F32, name="rs", tag="rs")
                nc.vector.reciprocal(rs[:], srt[:])
                xh = natp.tile([128, D], BF16, name="xh", tag="nat")
                nc.scalar.activation(xh[:], lt[:], AF.Copy, scale=rs[:])
                nc.sync.dma_start_transpose(xTv[:, :, t * 128:(t + 1) * 128], xh[:])

                pk = proj(t, wk_sb)
                knat = qknorm(pk)
                nc.sync.dma_start_transpose(KTv[:, :, t * 128:(t + 1) * 128],
                                            knat[:])
                pv = proj(t, wv_sb)
                nc.scalar.copy(vview[:, t, :, 0:HD],
                               pv[:].rearrange("p (h k) -> p h k", h=H))
                if t < NT_Q:
                    pq = proj(t, wq_sb)
                    qnat = qknorm(pq)
                    nc.sync.dma_start_transpose(QTv[:, :, t * 128:(t + 1) * 128],
                                                qnat[:])
            if exp_scale is None:
                for d in range(NT_D):
                    nc.vector.tensor_scalar_mul(
                        KTb[:, d * S:(d + 1) * S], KTb[:, d * S:(d + 1) * S],
                        kqc_t[:, d:d + 1])
        p_xT.release()

        # weight prefetch for phases D/E (DMA is idle during attention)
        wo_sb = [p_wo.tile([128, D], BF16, name=f"wo_sb{d}") for d in range(NT_D)]
        wi_sb = [p_wi.tile([128, MLP], BF16, name=f"wi_sb{d}") for d in range(NT_D)]
        for d in range(NT_D):
            nc.sync.dma_start(wo_sb[d][:], wo[d * 128:(d + 1) * 128, :])
            nc.sync.dma_start(wi_sb[d][:], wi[d * 128:(d + 1) * 128, :])

        # =============== Phase C: attention ===============
        es = 1.0 if exp_scale is None else exp_scale
        with ExitStack() as ctx:
            psL = ctx.enter_context(tc.tile_pool(name="c_psL", bufs=2, space="PSUM"))
            psO = ctx.enter_context(tc.tile_pool(name="c_psO", bufs=3, space="PSUM"))
            psB = ctx.enter_context(tc.tile_pool(name="c_psB", bufs=1, space="PSUM"))
            pp = ctx.enter_context(tc.tile_pool(name="c_p", bufs=3))
            oup = ctx.enter_context(tc.tile_pool(name="c_oU", bufs=9))
            dstp = ctx.enter_context(tc.tile_pool(name="c_dst", bufs=1))
            den = dstp.tile([H, SQ], F32, name="den")
            denr = dstp.tile([H, SQ], BF16, name="denr")
            dbp = ctx.enter_context(tc.tile_pool(name="c_db", bufs=4))

            def logits_mm(dt, t, qs, l_ps):
                for e in range(2):  # head 2*dt+e at PE row group 64*e
                    base = 64 * e
                    nc.tensor.matmul(
                        l_ps[:, e * CQ:(e + 1) * CQ],
                        KTb[base:base + 64, dt * S + t * 128:dt * S + (t + 1) * 128],
                        QTb[base:base + 64, dt * SQ + qs.start:dt * SQ + qs.stop],
                        start=True, stop=True)

            for j in range(SQ // CQ):
                qs = slice(j * CQ, (j + 1) * CQ)
                oUs = []
                for hp in range(H // 2):
                    dt = hp
                    o_ps = [psO.tile([VW, CQ], F32, name=f"o_ps{e}", tag="o_ps")
                            for e in range(2)]
                    l_ps = psL.tile([128, 2 * CQ], F32, name="l_ps", tag="l_ps")
                    logits_mm(dt, 0, qs, l_ps)
                    for t in range(NT_S):
                        l_nxt = None
                        if t + 1 < NT_S:
                            l_nxt = psL.tile([128, 2 * CQ], F32, name="l_ps",
                                             tag="l_ps")
                            logits_mm(dt, t + 1, qs, l_nxt)
                        p_t = pp.tile([128, 2 * CQ], BF16, name="p_t", tag="p_t")
                        nc.scalar.activation(p_t[:], l_ps[:], AF.Exp, scale=es)
                        for e in range(2):
                            h = 2 * hp + e
                            vofs = t * H * VW + h * VW
                            nc.tensor.matmul(
                                o_ps[e][:], Vaug[:, vofs:vofs + VW],
                                p_t[:, e * CQ:(e + 1) * CQ],
                                start=(t == 0), stop=(t == NT_S - 1))
                        l_ps = l_nxt
                    for e in range(2):
                        h = 2 * hp + e
                        oU = oup.tile([VW, CQ], F32, name="oU", tag="oU")
                        nc.vector.tensor_copy(oU[:], o_ps[e][:])
                        nc.sync.dma_start(den[h:h + 1, qs], oU[VW - 1:VW, :])
                        oUs.append((h, oU))
                    if hp % 3 == 2:   # divide heads in two batches of 6
                        h1 = oUs[-1][0] + 1   # start partition must be 0
                        with nc.allow_low_precision(reason="softmax denom recip"):
                            nc.vector.reciprocal(denr[0:h1, qs], den[0:h1, qs])
                        for h, oU in oUs:
                            dt, base = h // 2, (h % 2) * 64
                            db = dbp.tile([1, CQ], BF16, name="db", tag="db")
                            nc.sync.dma_start(db[:], denr[h:h + 1, qs])
                            b_ps = psB.tile([64, CQ], F32, name="b_ps", tag="b_ps")
                            nc.tensor.matmul(b_ps[:], onesB[0:1, :], db[:],
                                             start=True, stop=True)
                            nc.vector.scalar_tensor_tensor(
                                oT[base:base + 64,
                                   dt * SQ + j * CQ:dt * SQ + (j + 1) * CQ],
                                b_ps[:], 1.0, oU[0:HD, :],
                                op0=OP.bypass, op1=OP.mult)
                        oUs = []
        p_att.release()

        p_x2 = top.enter_context(tc.tile_pool(name="p_x2", bufs=1))
        x2 = [ptile(p_x2, [128, D], F32, name=f"x2_{q}") for q in range(NT_Q)]

        # =============== Phase D: out-proj + residual + ln2 ===============
        p_x2T = top.enter_context(tc.tile_pool(name="p_x2T", bufs=1))
        x2Tb = ptile(p_x2T, [128, NT_D * SQ], BF16, name="x2Tb")
        x2Tv = x2Tb[:].rearrange("p (d s) -> p d s", d=NT_D)
        p_w3 = top.enter_context(tc.tile_pool(name="p_w3", bufs=1))
        wom_sb = [p_w3.tile([128, D], BF16, name=f"wom_sb{m}") for m in range(NT_M)]
        for m in range(NT_M):
            nc.sync.dma_start(wom_sb[m][:], wom[m * 128:(m + 1) * 128, :])
        with ExitStack() as ctx:
            ps = ctx.enter_context(tc.tile_pool(name="d_ps", bufs=2, space="PSUM"))
            io = ctx.enter_context(tc.tile_pool(name="d_io", bufs=3))
            scr = ctx.enter_context(tc.tile_pool(name="d_scr", bufs=3))
            st_p = ctx.enter_context(tc.tile_pool(name="d_stats", bufs=8))

            for q in range(NT_Q):
                p = ps.tile([128, D], F32, name="p_oproj")
                for d in range(NT_D):
                    for ofs, n in _chunks(D):
                        nc.tensor.matmul(
                            p[:, ofs:ofs + n],
                            oT[:, d * SQ + q * 128: d * SQ + (q + 1) * 128],
                            wo_sb[d][:, ofs:ofs + n],
                            start=(d == 0), stop=(d == NT_D - 1))
                lt = io.tile([128, D], F32, name="lt_d")
                nc.sync.dma_start(lt[:], lat[q * 128:(q + 1) * 128, :])
                nc.vector.tensor_tensor(out=x2[q][:], in0=p[:], in1=lt[:], op=OP.add)
                sq = scr.tile([128, D], F32, name="sq_d")
                ssq = st_p.tile([128, 1], F32, name="ssq_d")
                nc.scalar.activation(sq[:], x2[q][:], AF.Square, accum_out=ssq[:])
                srt = st_p.tile([128, 1], F32, name="srt_d")
                nc.scalar.activation(srt[:], ssq[:], AF.Sqrt, bias=eps_t[:],
                                     scale=1.0 / D)
                rs = st_p.tile([128, 1], F32, name="rs_d")
                nc.vector.reciprocal(rs[:], srt[:])
                xh2 = scr.tile([128, D], BF16, name="xh2")
                nc.scalar.activation(xh2[:], x2[q][:], AF.Copy, scale=rs[:])
                nc.sync.dma_start_transpose(x2Tv[:, :, q * 128:(q + 1) * 128],
                                            xh2[:])
        # =============== Phase E: MLP ===============
        with ExitStack() as ctx:
            hTp = ctx.enter_context(tc.tile_pool(name="e_hT", bufs=1))
            ps = ctx.enter_context(tc.tile_pool(name="e_ps", bufs=1, space="PSUM"))
            iop = ctx.enter_context(tc.tile_pool(name="e_io", bufs=3))

            for j in range(SQ // CQ):
                hT = hTp.tile([128, NT_M * CQ], BF16, name="hT", tag="hT")
                for m in range(NT_M):
                    p = ps.tile([128, CQ], F32, name="p_mlp1", tag="p_mlp1", bufs=2)
                    for d in range(NT_D):
                        nc.tensor.matmul(
                            p[:],
                            wi_sb[d][:, m * 128:(m + 1) * 128],
                            x2Tb[:, d * SQ + j * CQ:d * SQ + (j + 1) * CQ],
                            start=(d == 0), stop=(d == NT_D - 1))
                    if not sim_compat:
                        nc.scalar.activation(hT[:, m * CQ:(m + 1) * CQ], p[:],
                                             AF.Gelu_apprx_tanh)
                    else:
                        xsq = iop.tile([128, CQ], F32, name="g_xsq", bufs=1)
                        nc.vector.tensor_tensor(out=xsq[:], in0=p[:], in1=p[:],
                                                op=OP.mult)
                        w = iop.tile([128, CQ], F32, name="g_w", bufs=1)
                        nc.vector.tensor_scalar(w[:], xsq[:], 0.044715, 1.0,
                                                op0=OP.mult, op1=OP.add)
                        u = iop.tile([128, CQ], F32, name="g_u", bufs=1)
                        nc.vector.tensor_tensor(out=u[:], in0=w[:], in1=p[:],
                                                op=OP.mult)
                        th = iop.tile([128, CQ], F32, name="g_th", bufs=1)
                        nc.scalar.activation(th[:], u[:], AF.Tanh,
                                             scale=0.7978845608028654)
                        t2 = iop.tile([128, CQ], F32, name="g_t2", bufs=1)
                        nc.vector.scalar_tensor_tensor(t2[:], th[:], 1.0, p[:],
                                                       op0=OP.add, op1=OP.mult)
                        nc.vector.tensor_scalar_mul(hT[:, m * CQ:(m + 1) * CQ],
                                                    t2[:], 0.5)

                for q in range(j * (NT_Q // 2), (j + 1) * (NT_Q // 2)):
                    qo = q * 128 - j * CQ
                    p = ps.tile([128, D], F32, name="p_mlp2", tag="p_mlp2", bufs=2)
                    for m in range(NT_M):
                        for ofs, n in _chunks(D):
                            nc.tensor.matmul(
                                p[:, ofs:ofs + n],
                                hT[:, m * CQ + qo: m * CQ + qo + 128],
                                wom_sb[m][:, ofs:ofs + n],
                                start=(m == 0), stop=(m == NT_M - 1))
                    ot = iop.tile([128, D], F32, name="ot_e")
                    nc.vector.tensor_tensor(out=ot[:], in0=p[:], in1=x2[q][:],
                                            op=OP.add)
                    nc.sync.dma_start(out[q * 128:(q + 1) * 128, :], ot[:])

    nc.compile()
    return nc


def _kq_vec(q_norm_scale, k_norm_scale):
    # per-channel scale for all D = H*HD projection channels
    return (np.tile(np.asarray(q_norm_scale, np.float64)
                    * np.asarray(k_norm_scale, np.float64), H)
            / np.sqrt(HD)).astype(np.float64)


def make_in_maps(latents, ln1_scale, wq, wk, wv, q_norm_scale, k_norm_scale,
                 wo_attn, ln2_scale, wi, wo_mlp):
    import ml_dtypes
    bf = ml_dtypes.bfloat16
    wq2 = (np.asarray(ln1_scale, np.float64)[:, None]
           * np.asarray(wq, np.float64).reshape(D, D)).astype(bf)
    wk2 = (np.asarray(ln1_scale, np.float64)[:, None]
           * np.asarray(wk, np.float64).reshape(D, D)).astype(bf)
    wv2 = (np.asarray(ln1_scale, np.float64)[:, None]
           * np.asarray(wv, np.float64).reshape(D, D)).astype(bf)
    wo2 = np.asarray(wo_attn, np.float32).reshape(D, D).astype(bf)
    wi2 = (np.asarray(ln2_scale, np.float64)[:, None]
           * np.asarray(wi, np.float64)).astype(bf)
    wom2 = np.asarray(wo_mlp, np.float32).astype(bf)
    kq = _kq_vec(q_norm_scale, k_norm_scale)     # [D] per-channel
    kqc = kq.reshape(NT_D, 128).T.astype(np.float32).copy()  # [128, NT_D]
    lat_np = np.asarray(latents, np.float32)
    in_maps = []
    for c in range(8):
        b, half = c // 2, c % 2
        lm = lat_np[b]
        lat_rot = np.concatenate([lm[half * SQ:(half + 1) * SQ],
                                  lm[(1 - half) * SQ:(2 - half) * SQ]], axis=0)
        in_maps.append(dict(lat=np.ascontiguousarray(lat_rot), wq=wq2, wk=wk2,
                            wv=wv2, wo=wo2, wi=wi2, wom=wom2, kqc=kqc))
    return in_maps


_NC_CACHE = {}


def get_nc(inputs, sim_compat=False):
    kq = _kq_vec(inputs["q_norm_scale"], inputs["k_norm_scale"])
    exp_scale = float(kq[0]) if np.allclose(kq, kq[0], rtol=1e-12) else None
    key = (exp_scale, sim_compat)
    if key not in _NC_CACHE:
        _NC_CACHE[key] = build_nc(exp_scale=exp_scale, sim_compat=sim_compat)
    return _NC_CACHE[key]


def kernel(**inputs):
    nc = get_nc(inputs)
    in_maps = make_in_maps(**inputs)
    res = run_bass_kernel_spmd(nc, in_maps, list(range(8)))
    y = np.empty((B, S, D), np.float32)
    for c in range(8):
        b, half = c // 2, c % 2
        y[b, half * SQ:(half + 1) * SQ] = res.results[c]["out"]
    return y


if __name__ == "__main__":
    import reference
    inputs = {k: np.asarray(v) for k, v in reference.setup_inputs().items()}
    y = kernel(**inputs)
    exp = np.asarray(reference.reference(**reference.setup_inputs()))
    err = np.abs(y - exp).max() / np.abs(exp).max()
    print("Relative error:", err)
